# revision 1
# baseline (speedup 1.0000x reference)
"""Binary-weight 3x3 conv2d (stride 1, VALID) on 8 Trainium2 NeuronCores.

Reference computes: out = conv2d(x, sign(weight)), NCHW/OIHW,
  x: (32, 128, 56, 56) f32, weight: (256, 128, 3, 3) f32 -> out (32, 256, 54, 54) f32.

Strategy:
  - Data-parallel over batch: 8 cores x 4 images each; weight replicated.
  - Conv as 9 shifted matmuls accumulated in PSUM (contraction over Cin=128 =
    partition dim). Weights are sign-binarized on device (+-1, exact in bf16).
  - fp32 accuracy at bf16 matmul speed: x is split into hi = bf16(x) and
    lo = bf16(x - hi); since w is +-1, products are exact and
    w@x ~= w@hi + w@lo to ~2^-17 relative, accumulated in fp32 PSUM.
  - Spatial tiling: 6 chunks of 9 output rows; each matmul streams a
    contiguous 9*56=504-wide window of the input row buffer, and the
    9x54 valid output columns are extracted on PSUM->SBUF eviction.
"""

import numpy as np
import concourse.bass as bass
import concourse.tile as tile
from concourse import bacc, mybir
from concourse import bass_utils

N_CORES = 8
CIN = 128
COUT = 256
H = W = 56
OH = OW = 54
HW = H * W          # 3136
OHW = OH * OW       # 2916
XPAD = 3200         # HW rounded up; pad covers reads up to 3138
ROWS_PER_CHUNK = 9
N_CHUNKS = OH // ROWS_PER_CHUNK   # 6
FREE = ROWS_PER_CHUNK * W         # 504 matmul free dim (<=512, one PSUM bank)
EV_FREE = ROWS_PER_CHUNK * OW     # 486 valid columns per chunk


def build_bass(n_imgs: int):
    f32, bf16 = mybir.dt.float32, mybir.dt.bfloat16
    nc = bacc.Bacc("TRN2", target_bir_lowering=False, debug=False,
                   num_devices=N_CORES)
    x_d = nc.dram_tensor("x", [n_imgs, CIN, HW], f32, kind="ExternalInput").ap()
    w_d = nc.dram_tensor("w", [CIN, 9 * COUT], f32, kind="ExternalInput").ap()
    out_d = nc.dram_tensor("out", [n_imgs, COUT, OHW], f32,
                           kind="ExternalOutput").ap()

    with tile.TileContext(nc) as tc:
        with (
            tc.tile_pool(name="wp", bufs=1) as wpool,
            tc.tile_pool(name="xp", bufs=2) as xpool,
            tc.tile_pool(name="hp", bufs=2) as hpool,
            tc.tile_pool(name="lp", bufs=2) as lpool,
            tc.tile_pool(name="op", bufs=4) as opool,
            tc.tile_pool(name="pp", bufs=4, space="PSUM") as pspool,
        ):
            wf = wpool.tile([CIN, 9 * COUT], f32)
            nc.sync.dma_start(wf[:], w_d[:])
            ws = wpool.tile([CIN, 9 * COUT], bf16)
            nc.scalar.sign(ws[:], wf[:])

            for n in range(n_imgs):
                xt = xpool.tile([CIN, XPAD], f32)
                nc.sync.dma_start(xt[:, :HW], x_d[n])
                hi = hpool.tile([CIN, XPAD], bf16)
                nc.scalar.copy(hi[:, :HW], xt[:, :HW])
                lo = lpool.tile([CIN, XPAD], bf16)
                nc.vector.tensor_sub(lo[:, :HW], xt[:, :HW], hi[:, :HW])
                # pad tails: keep matmul pad columns finite
                nc.scalar.memzero(hi[:, HW:])
                nc.vector.memset(lo[:, HW:], 0.0)

                for co in range(2):
                    for c in range(N_CHUNKS):
                        ps = pspool.tile([128, FREE], f32)
                        idx = 0
                        for kh in range(3):
                            for kw in range(3):
                                kcol = (kh * 3 + kw) * COUT + co * 128
                                lhsT = ws[:, kcol:kcol + 128]
                                off = (ROWS_PER_CHUNK * c + kh) * W + kw
                                for src in (hi, lo):
                                    nc.tensor.matmul(
                                        ps[:], lhsT, src[:, off:off + FREE],
                                        start=(idx == 0), stop=(idx == 17))
                                    idx += 1
                        ot = opool.tile([128, EV_FREE], f32)
                        ps3 = ps[:].rearrange("p (r w) -> p r w", w=W)[:, :, :OW]
                        ot3 = ot[:].rearrange("p (r w) -> p r w", w=OW)
                        if co == 0:
                            nc.vector.tensor_copy(ot3, ps3)
                        else:
                            nc.scalar.copy(ot3, ps3)
                        nc.sync.dma_start(
                            out_d[n, co * 128:(co + 1) * 128,
                                  EV_FREE * c:EV_FREE * (c + 1)],
                            ot[:])
    nc.compile()
    return nc


_NC_CACHE: dict[int, "bacc.Bacc"] = {}


def _get_nc(n_imgs: int):
    if n_imgs not in _NC_CACHE:
        _NC_CACHE[n_imgs] = build_bass(n_imgs)
    return _NC_CACHE[n_imgs]


def prep_weight(weight: np.ndarray) -> np.ndarray:
    # w_t[cin, (kh*3+kw)*256 + cout] = weight[cout, cin, kh, kw]
    return np.ascontiguousarray(
        weight.transpose(1, 2, 3, 0).reshape(CIN, 9 * COUT))


def run(x: np.ndarray, weight: np.ndarray, trace: bool = False):
    """Returns (out, BassKernelResults)."""
    x = np.ascontiguousarray(np.asarray(x, dtype=np.float32))
    weight = np.ascontiguousarray(np.asarray(weight, dtype=np.float32))
    n_total = x.shape[0]
    n_imgs = n_total // N_CORES
    w_t = prep_weight(weight)
    xs = x.reshape(N_CORES, n_imgs, CIN, HW)
    in_maps = [{"x": np.ascontiguousarray(xs[i]), "w": w_t}
               for i in range(N_CORES)]
    nc = _get_nc(n_imgs)
    res = bass_utils.run_bass_kernel_spmd(
        nc, in_maps, core_ids=list(range(N_CORES)), trace=trace)
    out = np.concatenate([res.results[i]["out"] for i in range(N_CORES)],
                         axis=0)
    return out.reshape(n_total, COUT, OH, OW), res


def kernel(x: np.ndarray, weight: np.ndarray) -> np.ndarray:
    out, _ = run(x, weight, trace=False)
    return out


if __name__ == "__main__":
    rng = np.random.default_rng(0)
    x = rng.standard_normal((32, CIN, H, W), dtype=np.float32)
    w = rng.standard_normal((COUT, CIN, 3, 3), dtype=np.float32)
    out = kernel(x, w)
    print(out.shape, out.dtype)


# revision 6
# speedup vs baseline: 1.0761x; 1.0761x over previous
"""Binary-weight 3x3 conv2d (stride 1, VALID) on 8 Trainium2 NeuronCores.

Reference computes: out = conv2d(x, sign(weight)), NCHW/OIHW,
  x: (32, 128, 56, 56) f32, weight: (256, 128, 3, 3) f32 -> out (32, 256, 54, 54) f32.

Strategy:
  - Data-parallel over batch: 8 cores x 4 images each; weight replicated.
  - Conv as 9 shifted matmuls accumulated in PSUM (contraction over Cin=128 =
    partition dim). Weights are sign-binarized on device (+-1, exact in bf16).
  - fp32 accuracy at bf16 matmul speed: x is split into hi = bf16(x) and
    lo = bf16(x - hi); since w is +-1, products are exact and
    w@x ~= w@hi + w@lo to ~2^-17 relative, accumulated in fp32 PSUM.
  - Spatial tiling: 6 chunks of 9 output rows; each matmul streams a
    contiguous 9*56=504-wide window of the input row buffer, and the
    9x54 valid output columns are extracted on PSUM->SBUF eviction.
"""

import numpy as np
import concourse.bass as bass
import concourse.tile as tile
from concourse import bacc, mybir
from concourse import bass_utils

N_CORES = 8
CIN = 128
COUT = 256
H = W = 56
OH = OW = 54
HW = H * W          # 3136
OHW = OH * OW       # 2916
ROWS_PER_CHUNK = 9
N_CHUNKS = OH // ROWS_PER_CHUNK   # 6
FREE = ROWS_PER_CHUNK * OW        # 486 matmul free dim (<=512, one PSUM bank)
# x rows are split in groups so the first matmuls can start before the
# whole image is hi/lo-split (prologue pipelining). Chunk c's matmuls read
# input rows [9c, 9c+11), so group boundaries are chosen to release chunk 0
# as early as possible.
ROW_GROUPS = ((0, 11), (11, 29), (29, 47), (47, 56))


def build_bass(n_imgs: int):
    f32, bf16 = mybir.dt.float32, mybir.dt.bfloat16
    nc = bacc.Bacc("TRN2", target_bir_lowering=False, debug=False,
                   num_devices=N_CORES)
    x_d = nc.dram_tensor("x", [n_imgs, CIN, HW], f32, kind="ExternalInput").ap()
    w_d = nc.dram_tensor("w", [CIN, 9 * COUT], f32, kind="ExternalInput").ap()
    out_d = nc.dram_tensor("out", [n_imgs, COUT, OHW], f32,
                           kind="ExternalOutput").ap()

    with tile.TileContext(nc) as tc:
        with (
            tc.tile_pool(name="wp", bufs=1) as wpool,
            tc.tile_pool(name="xp", bufs=2) as xpool,
            tc.tile_pool(name="hp", bufs=2) as hpool,
            tc.tile_pool(name="lp", bufs=2) as lpool,
            tc.tile_pool(name="op", bufs=4) as opool,
            tc.tile_pool(name="pp", bufs=4, space="PSUM") as pspool,
        ):
            # First weight block rides ahead of everything (its transfer is
            # what gates the first Ldweights); the remainder follows the first
            # x group. Sign is split so the first matmuls don't wait on the
            # whole weight.
            WSPLIT = 2 * COUT
            wf = wpool.tile([CIN, 9 * COUT], f32)
            ws = wpool.tile([CIN, 9 * COUT], bf16)
            nc.scalar.dma_start(wf[:, :WSPLIT], w_d[:, :WSPLIT])

            for n in range(n_imgs):
                xt = xpool.tile([CIN, HW], f32)
                hi = hpool.tile([CIN, HW], bf16)
                lo = lpool.tile([CIN, HW], bf16)
                for g, (r0, r1) in enumerate(ROW_GROUPS):
                    s = slice(r0 * W, r1 * W)
                    nc.sync.dma_start(xt[:, s], x_d[n, :, s])
                    if n == 0 and g == 0:
                        nc.scalar.sign(ws[:, :WSPLIT], wf[:, :WSPLIT])
                        nc.scalar.dma_start(wf[:, WSPLIT:], w_d[:, WSPLIT:])
                    nc.scalar.copy(hi[:, s], xt[:, s])
                    nc.vector.tensor_sub(lo[:, s], xt[:, s], hi[:, s])
                    if n == 0 and g == 1:
                        nc.scalar.sign(ws[:, WSPLIT:], wf[:, WSPLIT:])
                hi3 = hi[:].rearrange("p (r w) -> p r w", w=W)
                lo3 = lo[:].rearrange("p (r w) -> p r w", w=W)

                for co in range(2):
                    for c in range(N_CHUNKS):
                        r = ROWS_PER_CHUNK * c
                        ps = pspool.tile([128, FREE], f32)
                        idx = 0
                        for kh in range(3):
                            for kw in range(3):
                                kcol = (kh * 3 + kw) * COUT + co * 128
                                lhsT = ws[:, kcol:kcol + 128]
                                for src in (hi3, lo3):
                                    rhs = src[:, r + kh:r + kh + ROWS_PER_CHUNK,
                                              kw:kw + OW]
                                    nc.tensor.matmul(
                                        ps[:], lhsT, rhs,
                                        start=(idx == 0), stop=(idx == 17))
                                    idx += 1
                        ot = opool.tile([128, FREE], f32)
                        if co == 0:
                            nc.vector.tensor_copy(ot[:], ps[:])
                        else:
                            nc.scalar.copy(ot[:], ps[:])
                        nc.sync.dma_start(
                            out_d[n, co * 128:(co + 1) * 128,
                                  FREE * c:FREE * (c + 1)],
                            ot[:])
    nc.compile()
    return nc


_NC_CACHE: dict[int, "bacc.Bacc"] = {}


def _get_nc(n_imgs: int):
    if n_imgs not in _NC_CACHE:
        _NC_CACHE[n_imgs] = build_bass(n_imgs)
    return _NC_CACHE[n_imgs]


def prep_weight(weight: np.ndarray) -> np.ndarray:
    # w_t[cin, (kh*3+kw)*256 + cout] = weight[cout, cin, kh, kw]
    return np.ascontiguousarray(
        weight.transpose(1, 2, 3, 0).reshape(CIN, 9 * COUT))


def run(x: np.ndarray, weight: np.ndarray, trace: bool = False):
    """Returns (out, BassKernelResults)."""
    x = np.ascontiguousarray(np.asarray(x, dtype=np.float32))
    weight = np.ascontiguousarray(np.asarray(weight, dtype=np.float32))
    n_total = x.shape[0]
    n_imgs = n_total // N_CORES
    w_t = prep_weight(weight)
    xs = x.reshape(N_CORES, n_imgs, CIN, HW)
    in_maps = [{"x": np.ascontiguousarray(xs[i]), "w": w_t}
               for i in range(N_CORES)]
    nc = _get_nc(n_imgs)
    res = bass_utils.run_bass_kernel_spmd(
        nc, in_maps, core_ids=list(range(N_CORES)), trace=trace)
    out = np.concatenate([res.results[i]["out"] for i in range(N_CORES)],
                         axis=0)
    return out.reshape(n_total, COUT, OH, OW), res


def kernel(x: np.ndarray, weight: np.ndarray) -> np.ndarray:
    out, _ = run(x, weight, trace=False)
    return out


if __name__ == "__main__":
    rng = np.random.default_rng(0)
    x = rng.standard_normal((32, CIN, H, W), dtype=np.float32)
    w = rng.standard_normal((COUT, CIN, 3, 3), dtype=np.float32)
    out = kernel(x, w)
    print(out.shape, out.dtype)


# revision 9
# speedup vs baseline: 1.0897x; 1.0126x over previous
"""Binary-weight 3x3 conv2d (stride 1, VALID) on 8 Trainium2 NeuronCores.

Reference computes: out = conv2d(x, sign(weight)), NCHW/OIHW,
  x: (32, 128, 56, 56) f32, weight: (256, 128, 3, 3) f32 -> out (32, 256, 54, 54) f32.

Strategy:
  - Data-parallel over batch: 8 cores x 4 images each; weight replicated.
  - Conv as 9 shifted matmuls accumulated in PSUM (contraction over Cin=128 =
    partition dim). Weights are sign-binarized on device (+-1, exact in bf16).
  - fp32 accuracy at bf16 matmul speed: x is split into hi = bf16(x) and
    lo = bf16(x - hi); since w is +-1, products are exact and
    w@x ~= w@hi + w@lo to ~2^-17 relative, accumulated in fp32 PSUM.
  - Spatial tiling: 6 chunks of 9 output rows; each matmul streams a
    contiguous 9*56=504-wide window of the input row buffer, and the
    9x54 valid output columns are extracted on PSUM->SBUF eviction.
"""

import numpy as np
import concourse.bass as bass
import concourse.tile as tile
from concourse import bacc, mybir
from concourse import bass_utils

N_CORES = 8
CIN = 128
COUT = 256
H = W = 56
OH = OW = 54
HW = H * W          # 3136
OHW = OH * OW       # 2916
ROWS_PER_CHUNK = 9
N_CHUNKS = OH // ROWS_PER_CHUNK   # 6
FREE = ROWS_PER_CHUNK * OW        # 486 matmul free dim (<=512, one PSUM bank)
# x rows are split in groups so the first matmuls can start before the
# whole image is hi/lo-split (prologue pipelining). Chunk c's matmuls read
# input rows [9c, 9c+11), so group g releases chunk g-1 (and g=0 releases
# chunk 0) as early as possible.
ROW_GROUPS = ((0, 11), (11, 20), (20, 29), (29, 38), (38, 47), (47, 56))


def build_bass(n_imgs: int):
    f32, bf16 = mybir.dt.float32, mybir.dt.bfloat16
    nc = bacc.Bacc("TRN2", target_bir_lowering=False, debug=False,
                   num_devices=N_CORES)
    x_d = nc.dram_tensor("x", [n_imgs, CIN, HW], f32, kind="ExternalInput").ap()
    w_d = nc.dram_tensor("w", [CIN, 9 * COUT], f32, kind="ExternalInput").ap()
    out_d = nc.dram_tensor("out", [n_imgs, COUT, OHW], f32,
                           kind="ExternalOutput").ap()

    with tile.TileContext(nc) as tc:
        with (
            tc.tile_pool(name="wp", bufs=1) as wpool,
            tc.tile_pool(name="xp", bufs=2) as xpool,
            tc.tile_pool(name="hp", bufs=2) as hpool,
            tc.tile_pool(name="lp", bufs=2) as lpool,
            tc.tile_pool(name="op", bufs=4) as opool,
            tc.tile_pool(name="pp", bufs=4, space="PSUM") as pspool,
        ):
            # First weight block rides ahead of everything (its transfer is
            # what gates the first Ldweights); the remainder follows the first
            # x group. Sign is split so the first matmuls don't wait on the
            # whole weight.
            # Tap-aligned weight blocks: taps 0-1, 2-4, 5-8. Each sign fires
            # as soon as its block's DMA lands, just ahead of when the chunk-0
            # matmul stream consumes that tap's weights.
            WB = (0, 2 * COUT, 5 * COUT, 9 * COUT)
            wf = wpool.tile([CIN, 9 * COUT], f32)
            ws = wpool.tile([CIN, 9 * COUT], bf16)
            nc.scalar.dma_start(wf[:, WB[0]:WB[1]], w_d[:, WB[0]:WB[1]])

            # Warm the PE clock (HAM) with throwaway matmuls while the
            # prologue DMAs/splits run, so the real stream starts at full
            # clock. The dummy outputs are never read.
            warm = wpool.tile([128, 128], f32)
            nc.gpsimd.memset(warm[:], 0.0)
            for _ in range(6):
                wps = pspool.tile([128, 128], f32, name="wps", tag="warm_ps")
                nc.tensor.matmul(wps[:], warm[:], warm[:], start=True, stop=True)

            for n in range(n_imgs):
                xt = xpool.tile([CIN, HW], f32)
                hi = hpool.tile([CIN, HW], bf16)
                lo = lpool.tile([CIN, HW], bf16)
                for g, (r0, r1) in enumerate(ROW_GROUPS):
                    s = slice(r0 * W, r1 * W)
                    nc.sync.dma_start(xt[:, s], x_d[n, :, s])
                    if n == 0 and g == 0:
                        nc.scalar.sign(ws[:, WB[0]:WB[1]], wf[:, WB[0]:WB[1]])
                        nc.scalar.dma_start(wf[:, WB[1]:WB[2]], w_d[:, WB[1]:WB[2]])
                        nc.scalar.dma_start(wf[:, WB[2]:WB[3]], w_d[:, WB[2]:WB[3]])
                    if n == 0 and g == 0:
                        # keep the very first hi off the (busy) ACT queue
                        nc.vector.tensor_copy(hi[:, s], xt[:, s])
                    else:
                        nc.scalar.copy(hi[:, s], xt[:, s])
                    if n == 0 and g == 0:
                        nc.scalar.sign(ws[:, WB[1]:WB[2]], wf[:, WB[1]:WB[2]])
                    nc.vector.tensor_sub(lo[:, s], xt[:, s], hi[:, s])
                    if n == 0 and g == 1:
                        nc.scalar.sign(ws[:, WB[2]:WB[3]], wf[:, WB[2]:WB[3]])
                hi3 = hi[:].rearrange("p (r w) -> p r w", w=W)
                lo3 = lo[:].rearrange("p (r w) -> p r w", w=W)

                for co in range(2):
                    for c in range(N_CHUNKS):
                        r = ROWS_PER_CHUNK * c
                        ps = pspool.tile([128, FREE], f32)
                        idx = 0
                        for kh in range(3):
                            for kw in range(3):
                                kcol = (kh * 3 + kw) * COUT + co * 128
                                lhsT = ws[:, kcol:kcol + 128]
                                for src in (hi3, lo3):
                                    rhs = src[:, r + kh:r + kh + ROWS_PER_CHUNK,
                                              kw:kw + OW]
                                    nc.tensor.matmul(
                                        ps[:], lhsT, rhs,
                                        start=(idx == 0), stop=(idx == 17))
                                    idx += 1
                        ot = opool.tile([128, FREE], f32)
                        if co == 0:
                            nc.vector.tensor_copy(ot[:], ps[:])
                        else:
                            nc.scalar.copy(ot[:], ps[:])
                        nc.sync.dma_start(
                            out_d[n, co * 128:(co + 1) * 128,
                                  FREE * c:FREE * (c + 1)],
                            ot[:])
    nc.compile()
    return nc


_NC_CACHE: dict[int, "bacc.Bacc"] = {}


def _get_nc(n_imgs: int):
    if n_imgs not in _NC_CACHE:
        _NC_CACHE[n_imgs] = build_bass(n_imgs)
    return _NC_CACHE[n_imgs]


def prep_weight(weight: np.ndarray) -> np.ndarray:
    # w_t[cin, (kh*3+kw)*256 + cout] = weight[cout, cin, kh, kw]
    return np.ascontiguousarray(
        weight.transpose(1, 2, 3, 0).reshape(CIN, 9 * COUT))


def run(x: np.ndarray, weight: np.ndarray, trace: bool = False):
    """Returns (out, BassKernelResults)."""
    x = np.ascontiguousarray(np.asarray(x, dtype=np.float32))
    weight = np.ascontiguousarray(np.asarray(weight, dtype=np.float32))
    n_total = x.shape[0]
    n_imgs = n_total // N_CORES
    w_t = prep_weight(weight)
    xs = x.reshape(N_CORES, n_imgs, CIN, HW)
    in_maps = [{"x": np.ascontiguousarray(xs[i]), "w": w_t}
               for i in range(N_CORES)]
    nc = _get_nc(n_imgs)
    res = bass_utils.run_bass_kernel_spmd(
        nc, in_maps, core_ids=list(range(N_CORES)), trace=trace)
    out = np.concatenate([res.results[i]["out"] for i in range(N_CORES)],
                         axis=0)
    return out.reshape(n_total, COUT, OH, OW), res


def kernel(x: np.ndarray, weight: np.ndarray) -> np.ndarray:
    out, _ = run(x, weight, trace=False)
    return out


if __name__ == "__main__":
    rng = np.random.default_rng(0)
    x = rng.standard_normal((32, CIN, H, W), dtype=np.float32)
    w = rng.standard_normal((COUT, CIN, 3, 3), dtype=np.float32)
    out = kernel(x, w)
    print(out.shape, out.dtype)
